# revision 4
# baseline (speedup 1.0000x reference)
"""Trainium2 Bass kernel for nn_ConvLayerWithStyleMod.

Math: the reference is (per-sample style-modulated 3x3 conv_transpose stride 2)
followed by a fixed 4x4 blur. Both are convolutions, so they compose into a
single 6x6 kernel applied to the 2x-dilated input. Splitting the 256x256 output
by (row, col) parity turns that into FOUR dense 3x3 SAME convolutions on the
original 128x128 grid (in-ch 128 -> out-ch 64), which is pure TensorEngine work:
9 shifted matmuls per phase, phases paired two-at-a-time into M=128 matmuls.

Sharding: data-parallel over batch; sample i runs on core i (B == 8 == n_cores).
The tiny per-sample weight modulation/demodulation + blur composition is done
on host (float64) and shipped as a per-core weight tensor. x is shipped
zero-padded (130x130) so no on-device memsets are needed; everything the
matmul touches is float32r end to end (full-rate PE path).
"""

import math

import numpy as np

B, C, OC, SD, H = 8, 128, 64, 512, 128
KW = 3
EPS = 1e-8
HP = H + 2          # zero-padded image size in SBUF
NCORES = 8
RB = 4              # image rows per matmul chunk (N = RB*H = 512)
G = 8               # chunks in flight per group (= psum banks)

_prog = None


def _host_phase_weights(style, weight, mod_weight, mod_bias):
    """Per-sample phase kernels, laid out as matmul lhsT.

    Returns (B, C, 2*9*128) float32 where
    wts[b][c, (pc*9 + tap)*128 + pr*64 + o] = Kp[b, pr, pc, o, c, dr+1, dc+1],
    tap = (dr+1)*3 + (dc+1).
    """
    style = np.asarray(style, dtype=np.float64)
    weight = np.asarray(weight, dtype=np.float64)
    mod_weight = np.asarray(mod_weight, dtype=np.float64)
    mod_bias = np.asarray(mod_bias, dtype=np.float64)

    b = style.shape[0]
    scale = 1.0 / math.sqrt(C * KW * KW)
    w_gain = 1.0 / math.sqrt(mod_weight.shape[1])
    s = style @ (mod_weight.T * w_gain) + mod_bias            # (b, C)
    wmod = scale * weight * s[:, None, :, None, None]          # (b, OC, C, 3, 3)
    demod = 1.0 / np.sqrt(np.sum(wmod * wmod, axis=(2, 3, 4)) + EPS)
    wmod = wmod * demod[:, :, None, None, None]
    wt = wmod[:, :, :, ::-1, ::-1]

    fir = np.array([1.0, 3.0, 3.0, 1.0])
    kern = np.outer(fir, fir)
    kern = kern / kern.sum() * 4.0
    blurk = kern[::-1, ::-1]

    keff = np.zeros((b, OC, C, 6, 6))
    for u in range(3):
        for v in range(3):
            keff[:, :, :, u:u + 4, v:v + 4] += wt[:, :, :, u:u + 1, v:v + 1] * blurk

    rowsel = {0: slice(1, None, 2), 1: slice(0, None, 2)}
    wts = np.zeros((b, C, 2 * 9 * 128), dtype=np.float32)
    for pc in range(2):
        for pr in range(2):
            # (b, OC, C, 3, 3) for this phase
            kp = keff[:, :, :, rowsel[pr], :][:, :, :, :, rowsel[pc]]
            for dr in range(3):
                for dc in range(3):
                    tap = dr * 3 + dc
                    col0 = (pc * 9 + tap) * 128 + pr * 64
                    # [b, C, OC]
                    wts[:, :, col0:col0 + OC] = kp[:, :, :, dr, dc].transpose(0, 2, 1)
    return wts


def _build():
    import concourse.bacc as bacc
    import concourse.mybir as mybir
    from concourse.tile import TileContext

    f32 = mybir.dt.float32
    f32r = mybir.dt.float32r

    nc = bacc.Bacc(None, target_bir_lowering=False)
    x = nc.declare_dram_parameter("x", [C, HP * HP], f32r, isOutput=False)
    wts = nc.declare_dram_parameter("wts", [C, 2 * 9 * 128], f32r, isOutput=False)
    out = nc.declare_dram_parameter("out", [OC, 2 * H, 2 * H], f32, isOutput=True)

    with TileContext(nc) as tc:
        with (
            tc.tile_pool(name="xp", bufs=1) as xpool,
            tc.tile_pool(name="wp", bufs=1) as wpool,
            tc.tile_pool(name="ps", bufs=8, space="PSUM") as pspool,
            tc.tile_pool(name="ob", bufs=G + 2) as opool,
        ):
            xpad = xpool.tile([C, HP * HP], f32r)
            xv = xpad[:, :].rearrange("p (r c) -> p r c", c=HP)
            wtile = wpool.tile([C, 2 * 9 * 128], f32r)
            nc.sync.dma_start(out=wtile[:, :], in_=wts[:, :])

            # x arrives pre-padded; load in row-band slices so early chunks
            # can start compute before the whole image lands
            xdram = x.rearrange("p (r c) -> p r c", c=HP)
            NSLC = 32
            rs = HP // NSLC  # 16
            for sl in range(NSLC):
                r0 = sl * rs
                r1 = HP if sl == NSLC - 1 else r0 + rs
                nc.sync.dma_start(
                    out=xv[:, r0:r1, :],
                    in_=xdram[:, r0:r1, :],
                )

            nchunks = H // RB
            for g0 in range(0, nchunks, G):
                osb = [
                    opool.tile([C, RB, 2 * H], f32, tag="osb", name=f"osb{g0}_{i}")
                    for i in range(G)
                ]
                for pc in range(2):
                    ps = [
                        pspool.tile([C, RB, H], f32, tag="ps", name=f"ps{g0}_{pc}_{i}")
                        for i in range(G)
                    ]
                    for tap in range(9):
                        dr, dc = tap // 3 - 1, tap % 3 - 1
                        w_ap = wtile[:, (pc * 9 + tap) * 128:(pc * 9 + tap + 1) * 128]
                        for cg in range(G):
                            a0 = (g0 + cg) * RB
                            rhs = xv[:, a0 + dr + 1:a0 + dr + 1 + RB, 1 + dc:1 + dc + H]
                            nc.tensor.matmul(
                                ps[cg][:, :, :],
                                w_ap,
                                rhs,
                                start=(tap == 0),
                                stop=(tap == 8),
                            )
                    for cg in range(G):
                        nc.vector.tensor_copy(
                            out=osb[cg][:, :, pc::2], in_=ps[cg][:, :, :]
                        )
                for cg in range(G):
                    a0 = (g0 + cg) * RB
                    nc.sync.dma_start(
                        out=out[:, 2 * a0:2 * a0 + 2 * RB:2, :],
                        in_=osb[cg][0:OC, :, :],
                    )
                    nc.sync.dma_start(
                        out=out[:, 2 * a0 + 1:2 * a0 + 2 * RB:2, :],
                        in_=osb[cg][OC:2 * OC, :, :],
                    )
    nc.compile()
    return nc


def _get_prog():
    global _prog
    if _prog is None:
        _prog = _build()
    return _prog


def _pad_x(xi):
    xp = np.zeros((C, HP, HP), dtype=np.float32)
    xp[:, 1:1 + H, 1:1 + H] = xi
    return xp.reshape(C, HP * HP)


def kernel(x, style, weight, mod_weight, mod_bias):
    from concourse.bass_utils import run_bass_kernel_spmd

    nc = _get_prog()
    wts = _host_phase_weights(style, weight, mod_weight, mod_bias)
    x = np.asarray(x, dtype=np.float32)
    in_maps = [
        {"x": _pad_x(x[i]), "wts": np.ascontiguousarray(wts[i])}
        for i in range(NCORES)
    ]
    r = run_bass_kernel_spmd(nc, in_maps, list(range(NCORES)))
    return np.stack([r.results[i]["out"] for i in range(NCORES)], axis=0)
